# revision 13
# baseline (speedup 1.0000x reference)
"""Trainium2 Bass kernel for the GRU decoder (nn_Decoder_86397562126931).

Sharding (8 NeuronCores): GRU scan + attention replicated on all cores (the
128-step recurrence is weight-streaming bound on the PE array, so batch
parallelism cannot speed it up; replication avoids collectives), and the big
logits GEMM [2048,1024]@[1024,32000] tensor-parallel over the vocab dim:
core i computes vocab columns [i*4000,(i+1)*4000).

Device program (per core):
  A) layer-1 GRU scan, unit-major state; per step: h@W1h as f32r matmuls
     (out [16,3072] in PSUM), PE-transposes back to unit-major, gate math on
     DVE/ACT across all 128 lanes.  h history -> DRAM.
  B) x2 = rnn1 @ W2x + bias as one big transposed-out GEMM.
  C) layer-2 GRU scan (same as A).
  D) attention: qT GEMM, per-batch scores + softmax (+enc-mask bias), PE
     transpose, contextT GEMM (*dec-mask), last = tanh(concat @ Wc) -> DRAM.
  E) logits = last @ fcW_shard + fcb_shard -> output.

Host side only gathers W1x rows (one_hot @ W == row gather), folds biases,
and lays tensors out; all compute is on device.  Self-contained: shapes are
hardcoded.
"""

import sys

import numpy as np

for _p in ("/opt/trn_rl_repo",):
    if _p not in sys.path:
        sys.path.insert(0, _p)

B, TD, TE, U, V = 16, 128, 128, 1024, 32000
NCORES = 8
VS = V // NCORES          # vocab shard per core
TB = 16                   # scan steps per SBUF block
NEG = -1e9

_CACHE = {}


def _build(t_steps=TD, debug=False):
    import concourse.tile as tile
    from concourse import bacc, mybir
    from concourse.masks import make_identity
    from concourse.tile import add_dep_helper
    from contextlib import ExitStack

    f32 = mybir.dt.float32
    f32r = mybir.dt.float32r
    AF = mybir.ActivationFunctionType
    AX = mybir.AxisListType

    nbt = t_steps * 16
    nblk = t_steps // TB
    CHK = min(512, nbt)
    NCHK = nbt // CHK

    nc = bacc.Bacc("TRN2", target_bir_lowering=False, debug=False,
                   num_devices=NCORES)

    x1t_d = nc.dram_tensor("x1t", [24, 128, nbt], f32, kind="ExternalInput")
    w1h_d = nc.dram_tensor("w1h", [128, 8, 3072], f32r, kind="ExternalInput")
    w2x_d = nc.dram_tensor("w2x", [128, 8, 24, 128], f32r, kind="ExternalInput")
    w2h_d = nc.dram_tensor("w2h", [128, 8, 3072], f32r, kind="ExternalInput")
    b1h_d = nc.dram_tensor("b1h", [128, 128], f32, kind="ExternalInput")
    b2h_d = nc.dram_tensor("b2h", [128, 128], f32, kind="ExternalInput")
    b2ev_d = nc.dram_tensor("b2ev", [128, 24], f32, kind="ExternalInput")
    h0_d = nc.dram_tensor("h0", [128, 128], f32r, kind="ExternalInput")
    encn_d = nc.dram_tensor("encn", [16, 128, 1024], f32r, kind="ExternalInput")
    enct_d = nc.dram_tensor("enct", [16, 8, 128, 128], f32r, kind="ExternalInput")
    wa_d = nc.dram_tensor("wa", [128, 8, 8, 128], f32r, kind="ExternalInput")
    wc_d = nc.dram_tensor("wc", [128, 16, 8, 128], f32r, kind="ExternalInput")
    fcw_d = nc.dram_tensor("fcw", [128, 8, VS], f32r, kind="ExternalInput")
    fcb_d = nc.dram_tensor("fcb", [128, VS], f32, kind="ExternalInput")
    ebias_d = nc.dram_tensor("ebias", [16, 128, 128], f32, kind="ExternalInput")
    dfac_d = nc.dram_tensor("dfac", [16, 128, 128], f32, kind="ExternalInput")

    rnn1_d = nc.dram_tensor("rnn1", [8, 128, nbt], f32r)
    x2t_d = nc.dram_tensor("x2t", [24, 128, nbt], f32)
    rnn2_d = nc.dram_tensor("rnn2", [8, 128, nbt], f32r)
    last_d = nc.dram_tensor("lastd", [8, 128, nbt], f32r)

    # device-native layout [m, p=(t', b), v]; host unpermutes to [B, T, V]
    logit_d = nc.dram_tensor("logits", [t_steps * 16 // 128, 128, VS], f32,
                             kind="ExternalOutput")
    attn_d = nc.dram_tensor("attn", [B, t_steps, TE], f32,
                            kind="ExternalOutput")
    h1o_d = nc.dram_tensor("h1o", [B, U], f32, kind="ExternalOutput")
    h2o_d = nc.dram_tensor("h2o", [B, U], f32, kind="ExternalOutput")
    if debug:
        dbq_d = nc.dram_tensor("dbq", [128, 8, t_steps, 16], f32,
                               kind="ExternalOutput")
        dbs_d = nc.dram_tensor("dbs", [B, t_steps, 128], f32,
                               kind="ExternalOutput")
        dbc_d = nc.dram_tensor("dbc", [128, 8, t_steps, 16], f32,
                               kind="ExternalOutput")
        dbl_d = nc.dram_tensor("dbl", [8, 128, nbt], f32,
                               kind="ExternalOutput")

    CH_ORDER = (4, 5, 2, 3, 0, 1)  # h-cand chunks first, z last

    with tile.TileContext(nc) as tc:
        with ExitStack() as octx:
            const = octx.enter_context(tc.tile_pool(name="const", bufs=1))
            ident = const.tile([128, 128], f32)
            make_identity(nc, ident)
            h0_sb = const.tile([128, 8, 16], f32r)
            nc.sync.dma_start(h0_sb[:],
                              h0_d[:].rearrange("p (a b) -> p a b", a=8))

            def join_writes(writes):
                j = nc.sync.nop()
                for w in writes:
                    add_dep_helper(j.ins, w.ins, sync=True,
                                   reason="dram raw join")
                return j

            def dep_on(reader, j):
                add_dep_helper(reader.ins, j.ins, sync=True,
                               reason="dram raw read")
                return reader

            # ================= GRU scan (both layers) =====================
            def scan_layer(ctx, wh_d, xt_d, bh_d, hist_out_d, hout_d,
                           xt_join=None):
                wpool = ctx.enter_context(tc.tile_pool(name="w", bufs=1))
                wh_sb = wpool.tile([128, 8, 3072], f32r)
                nc.sync.dma_start(wh_sb[:], wh_d[:])
                bh_sb = wpool.tile([128, 8, 16], f32)
                nc.sync.dma_start(bh_sb[:],
                                  bh_d[:].rearrange("p (a b) -> p a b", a=8))

                xpool = ctx.enter_context(tc.tile_pool(name="x", bufs=2))
                hpool = ctx.enter_context(tc.tile_pool(name="h", bufs=2))
                cpool = ctx.enter_context(
                    tc.tile_pool(name="cp", bufs=3, space="PSUM"))
                zrp = ctx.enter_context(
                    tc.tile_pool(name="zrp", bufs=2, space="PSUM"))
                hhp = ctx.enter_context(
                    tc.tile_pool(name="hhp", bufs=2, space="PSUM"))
                spool = ctx.enter_context(tc.tile_pool(name="scp", bufs=4))
                gpool = ctx.enter_context(tc.tile_pool(name="g", bufs=2))

                prev_hist = None
                hist_writes = []
                for blk in range(nblk):
                    x_sb = xpool.tile([128, 24, TB * 16], f32)
                    xld = nc.sync.dma_start(
                        x_sb[:],
                        xt_d[:, :, blk * TB * 16:(blk + 1) * TB * 16]
                        .rearrange("m p n -> p m n"))
                    if xt_join is not None:
                        dep_on(xld, xt_join)
                    hist = hpool.tile([128, 8, TB * 16], f32r)
                    for t_ in range(TB):
                        if t_ == 0:
                            hp = (h0_sb[:] if blk == 0 else
                                  prev_hist[:, :, (TB - 1) * 16:TB * 16])
                        else:
                            hp = hist[:, :, (t_ - 1) * 16:t_ * 16]

                        zr_ps = zrp.tile([128, 16, 16], f32)
                        hh_ps = hhp.tile([128, 8, 16], f32)
                        for ci, c in enumerate(CH_ORDER):
                            ps = cpool.tile([16, 512], f32)
                            for kt in range(8):
                                nc.tensor.matmul(
                                    ps[:], hp[:, kt, :],
                                    wh_sb[:, kt, c * 512:(c + 1) * 512],
                                    start=(kt == 0), stop=(kt == 7))
                            cp = spool.tile([16, 512], f32)
                            if ci % 2 == 0:
                                nc.scalar.copy(cp[:], ps[:])
                            else:
                                nc.vector.tensor_copy(cp[:], ps[:])
                            for j in range(4):
                                uidx = c * 4 + j
                                dst = (hh_ps[:, uidx - 16, :] if uidx >= 16
                                       else zr_ps[:, uidx, :])
                                nc.tensor.transpose(
                                    dst, cp[:, j * 128:(j + 1) * 128],
                                    ident[:16, :16])

                        x_t = x_sb[:, :, t_ * 16:(t_ + 1) * 16]
                        zr_arg = gpool.tile([128, 16, 16], f32, tag="zra")
                        nc.vector.tensor_add(zr_arg[:], x_t[:, 0:16, :],
                                             zr_ps[:])
                        zr = gpool.tile([128, 16, 16], f32, tag="zr")
                        nc.scalar.activation(zr[:], zr_arg[:], AF.Sigmoid)
                        hhb = gpool.tile([128, 8, 16], f32, tag="hhb")
                        nc.vector.tensor_add(hhb[:], hh_ps[:], bh_sb[:])
                        rhh = gpool.tile([128, 8, 16], f32, tag="rhh")
                        nc.vector.tensor_mul(rhh[:], zr[:, 8:16, :], hhb[:])
                        ca = gpool.tile([128, 8, 16], f32, tag="ca")
                        nc.vector.tensor_add(ca[:], x_t[:, 16:24, :], rhh[:])
                        cand = gpool.tile([128, 8, 16], f32, tag="cand")
                        nc.scalar.activation(cand[:], ca[:], AF.Tanh)
                        d = gpool.tile([128, 8, 16], f32, tag="d")
                        nc.vector.tensor_sub(d[:], hp, cand[:])
                        zd = gpool.tile([128, 8, 16], f32, tag="zd")
                        nc.vector.tensor_mul(zd[:], zr[:, 0:8, :], d[:])
                        nc.vector.tensor_add(
                            hist[:, :, t_ * 16:(t_ + 1) * 16], cand[:], zd[:])

                    hist_writes.append(nc.sync.dma_start(
                        hist_out_d[:, :, blk * TB * 16:(blk + 1) * TB * 16]
                        .rearrange("k p n -> p k n"),
                        hist[:]))
                    prev_hist = hist

                hrow = wpool.tile([16, 1024], f32)
                for kt in range(8):
                    tmp = gpool.tile([128, 16], f32, tag="ftmp")
                    nc.vector.tensor_copy(
                        tmp[:], prev_hist[:, kt, (TB - 1) * 16:TB * 16])
                    tps = cpool.tile([16, 128], f32, tag="ps")
                    nc.tensor.transpose(tps[:], tmp[:], ident[:])
                    nc.vector.tensor_copy(
                        hrow[:, kt * 128:(kt + 1) * 128], tps[:])
                nc.sync.dma_start(hout_d[:], hrow[:])
                return join_writes(hist_writes)

            with ExitStack() as ctx:
                rnn1_join = scan_layer(ctx, w1h_d, x1t_d, b1h_d, rnn1_d,
                                       h1o_d)

            # ================= B: x2 = rnn1 @ W2x + bias ==================
            with ExitStack() as ctx:
                wp = ctx.enter_context(tc.tile_pool(name="bw", bufs=1))
                w2x_sb = wp.tile([128, 8, 24, 128], f32r)
                nc.sync.dma_start(w2x_sb[:], w2x_d[:])
                b2ev_sb = wp.tile([128, 24], f32)
                nc.sync.dma_start(b2ev_sb[:], b2ev_d[:])
                r1p = ctx.enter_context(tc.tile_pool(name="r1", bufs=1))
                r1_sb = r1p.tile([128, 8, nbt], f32r)
                dep_on(nc.sync.dma_start(
                    r1_sb[:], rnn1_d[:].rearrange("k p n -> p k n")),
                    rnn1_join)
                x2_writes = []
                psp = ctx.enter_context(
                    tc.tile_pool(name="bps", bufs=4, space="PSUM"))
                op = ctx.enter_context(tc.tile_pool(name="bo", bufs=4))
                for mt in range(24):
                    pss = []
                    for nb in range(NCHK):
                        ps = psp.tile([128, CHK], f32)
                        pss.append(ps)
                        for kt in range(8):
                            nc.tensor.matmul(
                                ps[:], w2x_sb[:, kt, mt, :],
                                r1_sb[:, kt, nb * CHK:(nb + 1) * CHK],
                                start=(kt == 0), stop=(kt == 7))
                    for nb, ps in enumerate(pss):
                        o = op.tile([128, CHK], f32)
                        if nb % 2 == 0:
                            nc.scalar.activation(o[:], ps[:], AF.Identity,
                                                 bias=b2ev_sb[:, mt:mt + 1])
                        else:
                            nc.vector.tensor_scalar_add(
                                o[:], ps[:], b2ev_sb[:, mt:mt + 1])
                        x2_writes.append(nc.sync.dma_start(
                            x2t_d[mt, :, nb * CHK:(nb + 1) * CHK], o[:]))

                x2_join = join_writes(x2_writes)

            # ================= C: layer-2 scan ============================
            with ExitStack() as ctx:
                rnn2_join = scan_layer(ctx, w2h_d, x2t_d, b2h_d, rnn2_d,
                                       h2o_d, xt_join=x2_join)

            # ================= D: attention ===============================
            with ExitStack() as ctx:
                r2p = ctx.enter_context(tc.tile_pool(name="r2", bufs=1))
                r2_sb = r2p.tile([128, 8, nbt], f32r)
                dep_on(nc.sync.dma_start(
                    r2_sb[:], rnn2_d[:].rearrange("k p n -> p k n")),
                    rnn2_join)
                last_writes = []
                bigp = ctx.enter_context(tc.tile_pool(name="big", bufs=1))

                with ExitStack() as c_at:
                    atp = c_at.enter_context(tc.tile_pool(name="atn", bufs=1))
                    attnT_all = atp.tile([128, 16, t_steps], f32r,
                                         tag="attnT")
                    if True:
                        qT_sb = bigp.tile([128, 8, 16, t_steps], f32r,
                                          tag="big")
                        with ExitStack() as c2:  # q = rnn2 @ Wa (transposed)
                            wap = c2.enter_context(
                                tc.tile_pool(name="wa", bufs=2))
                            qps = c2.enter_context(
                                tc.tile_pool(name="qps", bufs=4,
                                             space="PSUM"))
                            for mt in range(8):
                                wa_sb = wap.tile([128, 8, 128], f32r,
                                                 tag="wa")
                                nc.sync.dma_start(wa_sb[:],
                                                  wa_d[:, :, mt, :])
                                for nb in range(NCHK):
                                    ps = qps.tile([128, CHK], f32)
                                    for kt in range(8):
                                        nc.tensor.matmul(
                                            ps[:], wa_sb[:, kt, :],
                                            r2_sb[:, kt,
                                                  nb * CHK:(nb + 1) * CHK],
                                            start=(kt == 0), stop=(kt == 7))
                                    tchk = CHK // 16
                                    dst = qT_sb[:, mt, :,
                                                nb * tchk:(nb + 1) * tchk]
                                    src_ap = ps[:].rearrange(
                                        "p (t b) -> p b t", b=16)
                                    if mt % 2 == 0:
                                        nc.scalar.copy(dst, src_ap)
                                    else:
                                        nc.vector.tensor_copy(dst, src_ap)

                        with ExitStack() as c2:  # scores + softmax + attn
                            ep = c2.enter_context(
                                tc.tile_pool(name="enc", bufs=2))
                            mp = c2.enter_context(
                                tc.tile_pool(name="msk", bufs=2))
                            sps = c2.enter_context(
                                tc.tile_pool(name="sps", bufs=2,
                                             space="PSUM"))
                            tpp = c2.enter_context(
                                tc.tile_pool(name="tps", bufs=2,
                                             space="PSUM"))
                            sst = c2.enter_context(
                                tc.tile_pool(name="sst", bufs=3))
                            for b in range(B):
                                et = ep.tile([128, 8, 128], f32r, tag="enct")
                                nc.sync.dma_start(et[:], enct_d[b].rearrange('k p s -> p k s'))
                                eb = mp.tile([t_steps, 128], f32, tag="eb")
                                nc.sync.dma_start(eb[:],
                                                  ebias_d[b, :t_steps, :])
                                ps = sps.tile([t_steps, 128], f32)
                                for ut in range(8):
                                    nc.tensor.matmul(
                                        ps[:], qT_sb[:, ut, b, :],
                                        et[:, ut, :],
                                        start=(ut == 0), stop=(ut == 7))
                                scr = sst.tile([t_steps, 128], f32,
                                               tag="scr")
                                nc.vector.tensor_add(scr[:], ps[:], eb[:])
                                nmax = sst.tile([t_steps, 1], f32,
                                                tag="nmax")
                                nc.vector.reduce_max(nmax[:], scr[:],
                                                     axis=AX.X, negate=True)
                                esum = sst.tile([t_steps, 1], f32,
                                                tag="esum")
                                aexp = sst.tile([t_steps, 128], f32,
                                                tag="aexp")
                                nc.scalar.activation(aexp[:], scr[:], AF.Exp,
                                                     bias=nmax[:],
                                                     accum_out=esum[:])
                                rsum = sst.tile([t_steps, 1], f32,
                                                tag="rsum")
                                nc.vector.reciprocal(rsum[:], esum[:])
                                anorm = sst.tile([t_steps, 128], f32,
                                                 tag="anorm")
                                nc.vector.tensor_scalar_mul(anorm[:],
                                                            aexp[:],
                                                            rsum[:])
                                nc.sync.dma_start(attn_d[b], anorm[:])
                                tp = tpp.tile([128, t_steps], f32)
                                nc.tensor.transpose(
                                    tp[:], anorm[:],
                                    ident[:t_steps, :t_steps])
                                nc.vector.tensor_copy(attnT_all[:, b, :],
                                                      tp[:])
                                if debug:
                                    nc.sync.dma_start(dbs_d[b], scr[:])

                    if debug:
                        nc.sync.dma_start(dbq_d[:], qT_sb[:].bitcast(f32))
                    ctxT_sb = bigp.tile([128, 8, t_steps, 16], f32r,
                                        tag="big")
                    with ExitStack() as c2:  # contextT + dec mask
                        ep = c2.enter_context(
                            tc.tile_pool(name="encn", bufs=2))
                        mp = c2.enter_context(
                            tc.tile_pool(name="dmsk", bufs=2))
                        cps = c2.enter_context(
                            tc.tile_pool(name="cps", bufs=4, space="PSUM"))
                        for b in range(B):
                            en = ep.tile([128, 1024], f32r, tag="encn")
                            nc.sync.dma_start(en[:], encn_d[b])
                            df = mp.tile([128, t_steps], f32, tag="df")
                            nc.sync.dma_start(df[:], dfac_d[b, :, :t_steps])
                            for mt in range(8):
                                ps = cps.tile([128, t_steps], f32)
                                nc.tensor.matmul(
                                    ps[:], en[:, mt * 128:(mt + 1) * 128],
                                    attnT_all[:, b, :],
                                    start=True, stop=True)
                                nc.vector.tensor_mul(
                                    ctxT_sb[:, mt, :, b], ps[:], df[:])

                if debug:
                    nc.sync.dma_start(dbc_d[:], ctxT_sb[:].bitcast(f32))
                with ExitStack() as c2:  # last = tanh(concat @ Wc) -> DRAM
                    wcp = c2.enter_context(tc.tile_pool(name="wc", bufs=2))
                    lps = c2.enter_context(
                        tc.tile_pool(name="lps", bufs=4, space="PSUM"))
                    lop = c2.enter_context(tc.tile_pool(name="lo", bufs=3))
                    ctx_flat = ctxT_sb[:].rearrange("p a t b -> p a (t b)")
                    for mt in range(8):
                        wc_sb = wcp.tile([128, 16, 128], f32r, tag="wc")
                        nc.sync.dma_start(wc_sb[:], wc_d[:, :, mt, :])
                        pss = []
                        for nb in range(NCHK):
                            ps = lps.tile([128, CHK], f32)
                            pss.append(ps)
                            for kt in range(16):
                                rhs = (ctx_flat if kt < 8 else r2_sb[:])
                                ktt = kt % 8
                                nc.tensor.matmul(
                                    ps[:], wc_sb[:, kt, :],
                                    rhs[:, ktt, nb * CHK:(nb + 1) * CHK],
                                    start=(kt == 0), stop=(kt == 15))
                        for nb, ps in enumerate(pss):
                            o = lop.tile([128, CHK], f32r)
                            nc.scalar.activation(o[:], ps[:], AF.Tanh)
                            last_writes.append(nc.sync.dma_start(
                                last_d[mt, :, nb * CHK:(nb + 1) * CHK],
                                o[:]))

                last_join = join_writes(last_writes)
            if debug:
                dep_on(nc.sync.dma_start(dbl_d[:], last_d[:].bitcast(f32)),
                       last_join)
            # ================= E: logits = last @ fcW + fcb ===============
            with ExitStack() as ctx:
                fwp = ctx.enter_context(tc.tile_pool(name="fw", bufs=1))
                fcw_sb = fwp.tile([128, 8, VS], f32r)
                nc.sync.dma_start(fcw_sb[:], fcw_d[:])
                fcb_sb = fwp.tile([128, VS], f32)
                nc.sync.dma_start(fcb_sb[:], fcb_d[:])
                ltp = ctx.enter_context(tc.tile_pool(name="lt", bufs=16))
                fps = ctx.enter_context(
                    tc.tile_pool(name="fps", bufs=8, space="PSUM"))
                fop = ctx.enter_context(tc.tile_pool(name="fo", bufs=4))
                NV = VS // 500
                for m in range(nbt // 128):
                    lts = []
                    for kt in range(8):
                        lt = ltp.tile([128, 128], f32r, tag="lt")
                        dep_on(nc.sync.dma_start(
                            lt[:], last_d[kt, :, m * 128:(m + 1) * 128]),
                            last_join)
                        lts.append(lt)
                    pss = []
                    for nb in range(NV):
                        ps = fps.tile([128, 500], f32)
                        pss.append(ps)
                        for kt in range(8):
                            nc.tensor.matmul(
                                ps[:], lts[kt][:],
                                fcw_sb[:, kt, nb * 500:(nb + 1) * 500],
                                start=(kt == 0), stop=(kt == 7))
                    for nb, ps in enumerate(pss):
                        o = fop.tile([128, 500], f32)
                        nc.vector.tensor_add(
                            o[:], ps[:], fcb_sb[:, nb * 500:(nb + 1) * 500])
                        nc.sync.dma_start(
                            logit_d[m, :, nb * 500:(nb + 1) * 500], o[:])

    nc.compile()
    return nc


def _prep_host(inputs, t_steps=TD):
    tok = np.asarray(inputs["input_tokens"]).astype(np.int64)[:, :t_steps]
    dec_mask = np.asarray(inputs["dec_mask"]).astype(bool)
    enc = np.ascontiguousarray(np.asarray(inputs["enc_output"], np.float32))
    enc_mask = np.asarray(inputs["enc_mask"]).astype(bool)
    W1x = np.asarray(inputs["W1x"], np.float32)
    W1h = np.ascontiguousarray(np.asarray(inputs["W1h"], np.float32))
    b1 = np.asarray(inputs["b1"], np.float32)
    W2x = np.ascontiguousarray(np.asarray(inputs["W2x"], np.float32))
    W2h = np.ascontiguousarray(np.asarray(inputs["W2h"], np.float32))
    b2 = np.asarray(inputs["b2"], np.float32)
    Wa = np.ascontiguousarray(np.asarray(inputs["Wa"], np.float32))
    Wc = np.ascontiguousarray(np.asarray(inputs["Wc"], np.float32))
    fcW = np.ascontiguousarray(np.asarray(inputs["fcW"], np.float32))
    fcb = np.asarray(inputs["fcb"], np.float32)

    x1 = W1x[tok] + b1[0]                       # [B, t, 3U]
    x1[..., :2 * U] += b1[1][:2 * U]
    # [B,t,3U] -> [24 mt, 128 p, t*16 (t,b)]
    x1t = np.ascontiguousarray(
        x1.transpose(2, 1, 0).reshape(24, 128, t_steps * B))

    def ktile(w):  # [U, N] -> [128 p, U//128 kt, N]
        return np.ascontiguousarray(w.reshape(8, 128, -1).transpose(1, 0, 2))

    def lhstile(w, nmt):  # [K, M] -> [128 p, K//128 kt, nmt mt, 128 m]
        k = w.shape[0] // 128
        return np.ascontiguousarray(
            w.reshape(k, 128, nmt, 128).transpose(1, 0, 2, 3))

    def rep_bias(bh):  # [U] -> [128 p, 8 kt * 16 b] replicated over b
        return np.ascontiguousarray(
            np.repeat(bh.reshape(8, 128).T[:, :, None], 16, axis=2)
            .reshape(128, 128))

    b2c = b2[0].copy()
    b2c[:2 * U] += b2[1][:2 * U]

    eb = np.where(enc_mask, 0.0, np.float32(NEG)).astype(np.float32)
    ebias = np.ascontiguousarray(np.repeat(eb[:, None, :], 128, axis=1))
    df = dec_mask.astype(np.float32)
    dfac = np.ascontiguousarray(np.repeat(df[:, None, :], 128, axis=1))

    common = {
        "x1t": x1t,
        "w1h": ktile(W1h),
        "w2x": lhstile(W2x, 24),
        "w2h": ktile(W2h),
        "b1h": rep_bias(b1[1][2 * U:]),
        "b2h": rep_bias(b2[1][2 * U:]),
        "b2ev": np.ascontiguousarray(b2c.reshape(24, 128).T),
        "h0": np.zeros((128, 128), np.float32),
        "encn": enc,
        "enct": np.ascontiguousarray(
            enc.transpose(0, 2, 1).reshape(B, 8, 128, TE)),
        "wa": lhstile(Wa, 8),
        "wc": lhstile(Wc, 8),
        "ebias": ebias,
        "dfac": dfac,
    }
    in_maps = []
    for i in range(NCORES):
        m = dict(common)
        m["fcw"] = ktile(fcW[:, i * VS:(i + 1) * VS])
        m["fcb"] = np.ascontiguousarray(
            np.repeat(fcb[None, i * VS:(i + 1) * VS], 128, axis=0))
        in_maps.append(m)
    return in_maps


def _run(inputs, t_steps=TD, trace=False, debug=False):
    from concourse.bass_utils import run_bass_kernel_spmd

    key = ("nc", t_steps, debug)
    if key not in _CACHE:
        _CACHE[key] = _build(t_steps, debug=debug)
    nc = _CACHE[key]
    in_maps = _prep_host(inputs, t_steps)
    res = run_bass_kernel_spmd(nc, in_maps, core_ids=list(range(NCORES)),
                               trace=trace)

    def unperm(lg):  # [m, (t' b), v] -> [B, t, v]
        nm = lg.shape[0]
        return np.ascontiguousarray(
            lg.reshape(nm, 8, 16, VS).transpose(2, 0, 1, 3)
            .reshape(16, nm * 8, VS))

    logits = np.concatenate(
        [unperm(res.results[i]["logits"]) for i in range(NCORES)], axis=-1)
    attn = res.results[0]["attn"]
    h1 = res.results[0]["h1o"]
    h2 = res.results[0]["h2o"]
    return (logits, attn, h1, h2), res


def kernel(**inputs):
    out, _ = _run(inputs)
    return out


# revision 14
# speedup vs baseline: 1.0099x; 1.0099x over previous
"""Trainium2 Bass kernel for the GRU decoder (nn_Decoder_86397562126931).

Sharding (8 NeuronCores): GRU scan + attention replicated on all cores (the
128-step recurrence is weight-streaming bound on the PE array, so batch
parallelism cannot speed it up; replication avoids collectives), and the big
logits GEMM [2048,1024]@[1024,32000] tensor-parallel over the vocab dim:
core i computes vocab columns [i*4000,(i+1)*4000).

Device program (per core):
  A) layer-1 GRU scan, unit-major state; per step: h@W1h as f32r matmuls
     (out [16,3072] in PSUM), PE-transposes back to unit-major, gate math on
     DVE/ACT across all 128 lanes.  h history -> DRAM.
  B) x2 = rnn1 @ W2x + bias as one big transposed-out GEMM.
  C) layer-2 GRU scan (same as A).
  D) attention: qT GEMM, per-batch scores + softmax (+enc-mask bias), PE
     transpose, contextT GEMM (*dec-mask), last = tanh(concat @ Wc) -> DRAM.
  E) logits = last @ fcW_shard + fcb_shard -> output.

Host side only gathers W1x rows (one_hot @ W == row gather), folds biases,
and lays tensors out; all compute is on device.  Self-contained: shapes are
hardcoded.
"""

import sys

import numpy as np

for _p in ("/opt/trn_rl_repo",):
    if _p not in sys.path:
        sys.path.insert(0, _p)

B, TD, TE, U, V = 16, 128, 128, 1024, 32000
NCORES = 8
VS = V // NCORES          # vocab shard per core
TB = 16                   # scan steps per SBUF block
NEG = -1e9

_CACHE = {}


def _build(t_steps=TD, debug=False):
    import concourse.tile as tile
    from concourse import bacc, mybir
    from concourse.masks import make_identity
    from concourse.tile import add_dep_helper
    from contextlib import ExitStack

    f32 = mybir.dt.float32
    f32r = mybir.dt.float32r
    bf16 = mybir.dt.bfloat16
    AF = mybir.ActivationFunctionType
    AX = mybir.AxisListType

    nbt = t_steps * 16
    nblk = t_steps // TB
    CHK = min(512, nbt)
    NCHK = nbt // CHK

    nc = bacc.Bacc("TRN2", target_bir_lowering=False, debug=False,
                   num_devices=NCORES)

    x1t_d = nc.dram_tensor("x1t", [24, 128, nbt], f32, kind="ExternalInput")
    w1h_d = nc.dram_tensor("w1h", [128, 8, 3072], f32r, kind="ExternalInput")
    w2x_d = nc.dram_tensor("w2x", [128, 8, 24, 128], bf16, kind="ExternalInput")
    w2h_d = nc.dram_tensor("w2h", [128, 8, 3072], f32r, kind="ExternalInput")
    b1h_d = nc.dram_tensor("b1h", [16, 1024], f32, kind="ExternalInput")
    b2h_d = nc.dram_tensor("b2h", [16, 1024], f32, kind="ExternalInput")
    b2ev_d = nc.dram_tensor("b2ev", [128, 24], f32, kind="ExternalInput")
    h0_d = nc.dram_tensor("h0", [128, 128], f32r, kind="ExternalInput")
    encn_d = nc.dram_tensor("encn", [16, 128, 1024], f32r, kind="ExternalInput")
    enct_d = nc.dram_tensor("enct", [16, 8, 128, 128], f32r, kind="ExternalInput")
    wa_d = nc.dram_tensor("wa", [128, 8, 8, 128], f32r, kind="ExternalInput")
    wc_d = nc.dram_tensor("wc", [128, 16, 8, 128], bf16, kind="ExternalInput")
    fcw_d = nc.dram_tensor("fcw", [128, 8, VS], bf16, kind="ExternalInput")
    fcb_d = nc.dram_tensor("fcb", [128, VS], f32, kind="ExternalInput")
    ebias_d = nc.dram_tensor("ebias", [16, 128, 128], f32, kind="ExternalInput")
    dfac_d = nc.dram_tensor("dfac", [16, 128, 128], f32, kind="ExternalInput")

    rnn1_d = nc.dram_tensor("rnn1", [8, 128, nbt], f32r)
    x2t_d = nc.dram_tensor("x2t", [24, 128, nbt], f32)
    rnn2_d = nc.dram_tensor("rnn2", [8, 128, nbt], f32r)
    last_d = nc.dram_tensor("lastd", [8, 128, nbt], bf16)

    # device-native layout [m, p=(t', b), v]; host unpermutes to [B, T, V]
    logit_d = nc.dram_tensor("logits", [t_steps * 16 // 128, 128, VS], f32,
                             kind="ExternalOutput")
    attn_d = nc.dram_tensor("attn", [B, t_steps, TE], f32,
                            kind="ExternalOutput")
    h1o_d = nc.dram_tensor("h1o", [B, U], f32, kind="ExternalOutput")
    h2o_d = nc.dram_tensor("h2o", [B, U], f32, kind="ExternalOutput")
    if debug:
        dbq_d = nc.dram_tensor("dbq", [128, 8, t_steps, 16], f32,
                               kind="ExternalOutput")
        dbs_d = nc.dram_tensor("dbs", [B, t_steps, 128], f32,
                               kind="ExternalOutput")
        dbc_d = nc.dram_tensor("dbc", [128, 8, t_steps, 16], f32,
                               kind="ExternalOutput")
        dbl_d = nc.dram_tensor("dbl", [8, 128, nbt], f32,
                               kind="ExternalOutput")

    CH_ORDER = (4, 5, 2, 3, 0, 1)  # h-cand chunks first, z last

    with tile.TileContext(nc) as tc:
        with ExitStack() as octx:
            const = octx.enter_context(tc.tile_pool(name="const", bufs=1))
            ident = const.tile([128, 128], f32)
            make_identity(nc, ident)
            h0_sb = const.tile([128, 8, 16], f32r)
            nc.sync.dma_start(h0_sb[:],
                              h0_d[:].rearrange("p (a b) -> p a b", a=8))

            def join_writes(writes):
                j = nc.sync.nop()
                for w in writes:
                    add_dep_helper(j.ins, w.ins, sync=True,
                                   reason="dram raw join")
                return j

            def dep_on(reader, j):
                add_dep_helper(reader.ins, j.ins, sync=True,
                               reason="dram raw read")
                return reader

            # ================= GRU scan (both layers) =====================
            def scan_layer(ctx, wh_d, xt_d, bh_d, hist_out_d, hout_d,
                           xt_join=None):
                wpool = ctx.enter_context(tc.tile_pool(name="w", bufs=1))
                wh_sb = wpool.tile([128, 8, 3072], f32r)
                nc.sync.dma_start(wh_sb[:], wh_d[:])
                bh_sb = wpool.tile([16, 1024], f32)
                nc.sync.dma_start(bh_sb[:], bh_d[:])

                xpool = ctx.enter_context(tc.tile_pool(name="x", bufs=2))
                hpool = ctx.enter_context(tc.tile_pool(name="h", bufs=2))
                cpool = ctx.enter_context(
                    tc.tile_pool(name="cp", bufs=4, space="PSUM"))
                zrp = ctx.enter_context(
                    tc.tile_pool(name="zrp", bufs=2, space="PSUM"))
                hhp = ctx.enter_context(
                    tc.tile_pool(name="hhp", bufs=2, space="PSUM"))
                spool = ctx.enter_context(tc.tile_pool(name="scp", bufs=4))
                gpool = ctx.enter_context(tc.tile_pool(name="g", bufs=2))

                prev_hist = None
                hist_writes = []
                for blk in range(nblk):
                    x_sb = xpool.tile([128, 24, TB * 16], f32)
                    xld = nc.sync.dma_start(
                        x_sb[:],
                        xt_d[:, :, blk * TB * 16:(blk + 1) * TB * 16]
                        .rearrange("m p n -> p m n"))
                    if xt_join is not None:
                        dep_on(xld, xt_join)
                    hist = hpool.tile([128, 8, TB * 16], f32r)
                    for t_ in range(TB):
                        if t_ == 0:
                            hp = (h0_sb[:] if blk == 0 else
                                  prev_hist[:, :, (TB - 1) * 16:TB * 16])
                        else:
                            hp = hist[:, :, (t_ - 1) * 16:t_ * 16]

                        zr_ps = zrp.tile([128, 16, 16], f32)
                        hh_ps = hhp.tile([128, 8, 16], f32)
                        for ci, c in enumerate(CH_ORDER):
                            ps = cpool.tile([16, 512], f32)
                            for kt in range(8):
                                nc.tensor.matmul(
                                    ps[:], hp[:, kt, :],
                                    wh_sb[:, kt, c * 512:(c + 1) * 512],
                                    start=(kt == 0), stop=(kt == 7))
                            cp = spool.tile([16, 512], f32)
                            if c >= 4:  # hh chunks: fold in recurrent bias
                                nc.vector.tensor_add(
                                    cp[:], ps[:],
                                    bh_sb[:, (c - 4) * 512:(c - 3) * 512])
                            elif ci % 2 == 0:
                                nc.scalar.copy(cp[:], ps[:])
                            else:
                                nc.vector.tensor_copy(cp[:], ps[:])
                            for j in range(4):
                                uidx = c * 4 + j
                                dst = (hh_ps[:, uidx - 16, :] if uidx >= 16
                                       else zr_ps[:, uidx, :])
                                nc.tensor.transpose(
                                    dst, cp[:, j * 128:(j + 1) * 128],
                                    ident[:16, :16])

                        x_t = x_sb[:, :, t_ * 16:(t_ + 1) * 16]
                        zr_arg = gpool.tile([128, 16, 16], f32, tag="zra")
                        nc.vector.tensor_add(zr_arg[:], x_t[:, 0:16, :],
                                             zr_ps[:])
                        zr = gpool.tile([128, 16, 16], f32, tag="zr")
                        nc.scalar.activation(zr[:], zr_arg[:], AF.Sigmoid)
                        omz = gpool.tile([128, 8, 16], f32, tag="omz")
                        nc.scalar.activation(omz[:], zr_arg[:, 0:8, :],
                                             AF.Sigmoid, scale=-1.0)
                        zh = gpool.tile([128, 8, 16], f32, tag="zh")
                        nc.vector.tensor_mul(zh[:], zr[:, 0:8, :], hp)
                        rhh = gpool.tile([128, 8, 16], f32, tag="rhh")
                        nc.vector.tensor_mul(rhh[:], zr[:, 8:16, :], hh_ps[:])
                        ca = gpool.tile([128, 8, 16], f32, tag="ca")
                        nc.vector.tensor_add(ca[:], x_t[:, 16:24, :], rhh[:])
                        cand = gpool.tile([128, 8, 16], f32, tag="cand")
                        nc.scalar.activation(cand[:], ca[:], AF.Tanh)
                        oc = gpool.tile([128, 8, 16], f32, tag="oc")
                        nc.vector.tensor_mul(oc[:], omz[:], cand[:])
                        nc.vector.tensor_add(
                            hist[:, :, t_ * 16:(t_ + 1) * 16], zh[:], oc[:])

                    hist_writes.append(nc.sync.dma_start(
                        hist_out_d[:, :, blk * TB * 16:(blk + 1) * TB * 16]
                        .rearrange("k p n -> p k n"),
                        hist[:]))
                    prev_hist = hist

                hrow = wpool.tile([16, 1024], f32)
                for kt in range(8):
                    tmp = gpool.tile([128, 16], f32, tag="ftmp")
                    nc.vector.tensor_copy(
                        tmp[:], prev_hist[:, kt, (TB - 1) * 16:TB * 16])
                    tps = cpool.tile([16, 128], f32, tag="ps")
                    nc.tensor.transpose(tps[:], tmp[:], ident[:])
                    nc.vector.tensor_copy(
                        hrow[:, kt * 128:(kt + 1) * 128], tps[:])
                nc.sync.dma_start(hout_d[:], hrow[:])
                return join_writes(hist_writes)

            with ExitStack() as ctx:
                rnn1_join = scan_layer(ctx, w1h_d, x1t_d, b1h_d, rnn1_d,
                                       h1o_d)

            # ================= B: x2 = rnn1 @ W2x + bias ==================
            with ExitStack() as ctx:
                wp = ctx.enter_context(tc.tile_pool(name="bw", bufs=1))
                w2x_sb = wp.tile([128, 8, 24, 128], bf16)
                nc.sync.dma_start(w2x_sb[:], w2x_d[:])
                b2ev_sb = wp.tile([128, 24], f32)
                nc.sync.dma_start(b2ev_sb[:], b2ev_d[:])
                r1p = ctx.enter_context(tc.tile_pool(name="r1", bufs=1))
                r1_sb = r1p.tile([128, 8, nbt], f32r)
                dep_on(nc.sync.dma_start(
                    r1_sb[:], rnn1_d[:].rearrange("k p n -> p k n")),
                    rnn1_join)
                r1b_sb = r1p.tile([128, 8, nbt], bf16)
                for kt in range(8):
                    nc.vector.tensor_copy(r1b_sb[:, kt, :], r1_sb[:, kt, :])
                x2_writes = []
                psp = ctx.enter_context(
                    tc.tile_pool(name="bps", bufs=4, space="PSUM"))
                op = ctx.enter_context(tc.tile_pool(name="bo", bufs=4))
                for mt in range(24):
                    pss = []
                    for nb in range(NCHK):
                        ps = psp.tile([128, CHK], f32)
                        pss.append(ps)
                        for kt in range(8):
                            nc.tensor.matmul(
                                ps[:], w2x_sb[:, kt, mt, :],
                                r1b_sb[:, kt, nb * CHK:(nb + 1) * CHK],
                                start=(kt == 0), stop=(kt == 7))
                    for nb, ps in enumerate(pss):
                        o = op.tile([128, CHK], f32)
                        if nb % 2 == 0:
                            nc.scalar.activation(o[:], ps[:], AF.Identity,
                                                 bias=b2ev_sb[:, mt:mt + 1])
                        else:
                            nc.vector.tensor_scalar_add(
                                o[:], ps[:], b2ev_sb[:, mt:mt + 1])
                        x2_writes.append(nc.sync.dma_start(
                            x2t_d[mt, :, nb * CHK:(nb + 1) * CHK], o[:]))

                x2_join = join_writes(x2_writes)

            # ================= C: layer-2 scan ============================
            with ExitStack() as ctx:
                rnn2_join = scan_layer(ctx, w2h_d, x2t_d, b2h_d, rnn2_d,
                                       h2o_d, xt_join=x2_join)

            # ================= D: attention ===============================
            with ExitStack() as ctx:
                r2p = ctx.enter_context(tc.tile_pool(name="r2", bufs=1))
                r2_sb = r2p.tile([128, 8, nbt], f32r)
                dep_on(nc.sync.dma_start(
                    r2_sb[:], rnn2_d[:].rearrange("k p n -> p k n")),
                    rnn2_join)
                r2b_sb = r2p.tile([128, 8, nbt], bf16)
                for kt in range(8):
                    nc.vector.tensor_copy(r2b_sb[:, kt, :], r2_sb[:, kt, :])
                last_writes = []
                bigp = ctx.enter_context(tc.tile_pool(name="big", bufs=1))

                with ExitStack() as c_at:
                    atp = c_at.enter_context(tc.tile_pool(name="atn", bufs=1))
                    attnT_all = atp.tile([128, 16, t_steps], f32r,
                                         tag="attnT")
                    if True:
                        qT_sb = bigp.tile([128, 8, 16, t_steps], f32r,
                                          tag="big")
                        with ExitStack() as c2:  # q = rnn2 @ Wa (transposed)
                            wap = c2.enter_context(
                                tc.tile_pool(name="wa", bufs=2))
                            qps = c2.enter_context(
                                tc.tile_pool(name="qps", bufs=4,
                                             space="PSUM"))
                            for mt in range(8):
                                wa_sb = wap.tile([128, 8, 128], f32r,
                                                 tag="wa")
                                nc.sync.dma_start(wa_sb[:],
                                                  wa_d[:, :, mt, :])
                                for nb in range(NCHK):
                                    ps = qps.tile([128, CHK], f32)
                                    for kt in range(8):
                                        nc.tensor.matmul(
                                            ps[:], wa_sb[:, kt, :],
                                            r2_sb[:, kt,
                                                  nb * CHK:(nb + 1) * CHK],
                                            start=(kt == 0), stop=(kt == 7))
                                    tchk = CHK // 16
                                    dst = qT_sb[:, mt, :,
                                                nb * tchk:(nb + 1) * tchk]
                                    src_ap = ps[:].rearrange(
                                        "p (t b) -> p b t", b=16)
                                    if mt % 2 == 0:
                                        nc.scalar.copy(dst, src_ap)
                                    else:
                                        nc.vector.tensor_copy(dst, src_ap)

                        with ExitStack() as c2:  # scores + softmax + attn
                            ep = c2.enter_context(
                                tc.tile_pool(name="enc", bufs=2))
                            mp = c2.enter_context(
                                tc.tile_pool(name="msk", bufs=2))
                            sps = c2.enter_context(
                                tc.tile_pool(name="sps", bufs=2,
                                             space="PSUM"))
                            tpp = c2.enter_context(
                                tc.tile_pool(name="tps", bufs=2,
                                             space="PSUM"))
                            sst = c2.enter_context(
                                tc.tile_pool(name="sst", bufs=3))
                            for b in range(B):
                                et = ep.tile([128, 8, 128], f32r, tag="enct")
                                nc.sync.dma_start(et[:], enct_d[b].rearrange('k p s -> p k s'))
                                eb = mp.tile([t_steps, 128], f32, tag="eb")
                                nc.sync.dma_start(eb[:],
                                                  ebias_d[b, :t_steps, :])
                                ps = sps.tile([t_steps, 128], f32)
                                for ut in range(8):
                                    nc.tensor.matmul(
                                        ps[:], qT_sb[:, ut, b, :],
                                        et[:, ut, :],
                                        start=(ut == 0), stop=(ut == 7))
                                scr = sst.tile([t_steps, 128], f32,
                                               tag="scr")
                                nc.vector.tensor_add(scr[:], ps[:], eb[:])
                                nmax = sst.tile([t_steps, 1], f32,
                                                tag="nmax")
                                nc.vector.reduce_max(nmax[:], scr[:],
                                                     axis=AX.X, negate=True)
                                esum = sst.tile([t_steps, 1], f32,
                                                tag="esum")
                                aexp = sst.tile([t_steps, 128], f32,
                                                tag="aexp")
                                nc.scalar.activation(aexp[:], scr[:], AF.Exp,
                                                     bias=nmax[:],
                                                     accum_out=esum[:])
                                rsum = sst.tile([t_steps, 1], f32,
                                                tag="rsum")
                                nc.vector.reciprocal(rsum[:], esum[:])
                                anorm = sst.tile([t_steps, 128], f32,
                                                 tag="anorm")
                                nc.vector.tensor_scalar_mul(anorm[:],
                                                            aexp[:],
                                                            rsum[:])
                                nc.sync.dma_start(attn_d[b], anorm[:])
                                tp = tpp.tile([128, t_steps], f32)
                                nc.tensor.transpose(
                                    tp[:], anorm[:],
                                    ident[:t_steps, :t_steps])
                                nc.vector.tensor_copy(attnT_all[:, b, :],
                                                      tp[:])
                                if debug:
                                    nc.sync.dma_start(dbs_d[b], scr[:])

                    if debug:
                        nc.sync.dma_start(dbq_d[:], qT_sb[:].bitcast(f32))
                    ctxT_sb = bigp.tile([128, 8, t_steps, 16], bf16,
                                        tag="big")
                    with ExitStack() as c2:  # contextT + dec mask
                        ep = c2.enter_context(
                            tc.tile_pool(name="encn", bufs=2))
                        mp = c2.enter_context(
                            tc.tile_pool(name="dmsk", bufs=2))
                        cps = c2.enter_context(
                            tc.tile_pool(name="cps", bufs=4, space="PSUM"))
                        for b in range(B):
                            en = ep.tile([128, 1024], f32r, tag="encn")
                            nc.sync.dma_start(en[:], encn_d[b])
                            df = mp.tile([128, t_steps], f32, tag="df")
                            nc.sync.dma_start(df[:], dfac_d[b, :, :t_steps])
                            for mt in range(8):
                                ps = cps.tile([128, t_steps], f32)
                                nc.tensor.matmul(
                                    ps[:], en[:, mt * 128:(mt + 1) * 128],
                                    attnT_all[:, b, :],
                                    start=True, stop=True)
                                nc.vector.tensor_mul(
                                    ctxT_sb[:, mt, :, b], ps[:], df[:])

                if debug:
                    nc.sync.dma_start(dbc_d[:], ctxT_sb[:].bitcast(f32))
                with ExitStack() as c2:  # last = tanh(concat @ Wc) -> DRAM
                    wcp = c2.enter_context(tc.tile_pool(name="wc", bufs=2))
                    lps = c2.enter_context(
                        tc.tile_pool(name="lps", bufs=4, space="PSUM"))
                    lop = c2.enter_context(tc.tile_pool(name="lo", bufs=3))
                    ctx_flat = ctxT_sb[:].rearrange("p a t b -> p a (t b)")
                    for mt in range(8):
                        wc_sb = wcp.tile([128, 16, 128], bf16, tag="wc")
                        nc.sync.dma_start(wc_sb[:], wc_d[:, :, mt, :])
                        pss = []
                        for nb in range(NCHK):
                            ps = lps.tile([128, CHK], f32)
                            pss.append(ps)
                            for kt in range(16):
                                rhs = (ctx_flat if kt < 8 else r2b_sb[:])
                                ktt = kt % 8
                                nc.tensor.matmul(
                                    ps[:], wc_sb[:, kt, :],
                                    rhs[:, ktt, nb * CHK:(nb + 1) * CHK],
                                    start=(kt == 0), stop=(kt == 15))
                        for nb, ps in enumerate(pss):
                            o = lop.tile([128, CHK], bf16)
                            nc.scalar.activation(o[:], ps[:], AF.Tanh)
                            last_writes.append(nc.sync.dma_start(
                                last_d[mt, :, nb * CHK:(nb + 1) * CHK],
                                o[:]))

                last_join = join_writes(last_writes)
            if debug:
                dep_on(nc.gpsimd.dma_start(dbl_d[:], last_d[:]),
                       last_join)
            # ================= E: logits = last @ fcW + fcb ===============
            with ExitStack() as ctx:
                fwp = ctx.enter_context(tc.tile_pool(name="fw", bufs=1))
                fcw_sb = fwp.tile([128, 8, VS], bf16)
                nc.sync.dma_start(fcw_sb[:], fcw_d[:])
                fcb_sb = fwp.tile([128, VS], f32)
                nc.sync.dma_start(fcb_sb[:], fcb_d[:])
                ltp = ctx.enter_context(tc.tile_pool(name="lt", bufs=16))
                fps = ctx.enter_context(
                    tc.tile_pool(name="fps", bufs=8, space="PSUM"))
                fop = ctx.enter_context(tc.tile_pool(name="fo", bufs=4))
                NV = VS // 500
                for m in range(nbt // 128):
                    lts = []
                    for kt in range(8):
                        lt = ltp.tile([128, 128], bf16, tag="lt")
                        dep_on(nc.sync.dma_start(
                            lt[:], last_d[kt, :, m * 128:(m + 1) * 128]),
                            last_join)
                        lts.append(lt)
                    pss = []
                    for nb in range(NV):
                        ps = fps.tile([128, 500], f32)
                        pss.append(ps)
                        for kt in range(8):
                            nc.tensor.matmul(
                                ps[:], lts[kt][:],
                                fcw_sb[:, kt, nb * 500:(nb + 1) * 500],
                                start=(kt == 0), stop=(kt == 7))
                    for nb, ps in enumerate(pss):
                        o = fop.tile([128, 500], f32)
                        nc.vector.tensor_add(
                            o[:], ps[:], fcb_sb[:, nb * 500:(nb + 1) * 500])
                        nc.sync.dma_start(
                            logit_d[m, :, nb * 500:(nb + 1) * 500], o[:])

    nc.compile()
    return nc


def _prep_host(inputs, t_steps=TD):
    import ml_dtypes
    bf = ml_dtypes.bfloat16
    tok = np.asarray(inputs["input_tokens"]).astype(np.int64)[:, :t_steps]
    dec_mask = np.asarray(inputs["dec_mask"]).astype(bool)
    enc = np.ascontiguousarray(np.asarray(inputs["enc_output"], np.float32))
    enc_mask = np.asarray(inputs["enc_mask"]).astype(bool)
    W1x = np.asarray(inputs["W1x"], np.float32)
    W1h = np.ascontiguousarray(np.asarray(inputs["W1h"], np.float32))
    b1 = np.asarray(inputs["b1"], np.float32)
    W2x = np.ascontiguousarray(np.asarray(inputs["W2x"], np.float32))
    W2h = np.ascontiguousarray(np.asarray(inputs["W2h"], np.float32))
    b2 = np.asarray(inputs["b2"], np.float32)
    Wa = np.ascontiguousarray(np.asarray(inputs["Wa"], np.float32))
    Wc = np.ascontiguousarray(np.asarray(inputs["Wc"], np.float32))
    fcW = np.ascontiguousarray(np.asarray(inputs["fcW"], np.float32))
    fcb = np.asarray(inputs["fcb"], np.float32)

    x1 = W1x[tok] + b1[0]                       # [B, t, 3U]
    x1[..., :2 * U] += b1[1][:2 * U]
    # [B,t,3U] -> [24 mt, 128 p, t*16 (t,b)]
    x1t = np.ascontiguousarray(
        x1.transpose(2, 1, 0).reshape(24, 128, t_steps * B))

    def ktile(w):  # [U, N] -> [128 p, U//128 kt, N]
        return np.ascontiguousarray(w.reshape(8, 128, -1).transpose(1, 0, 2))

    def lhstile(w, nmt):  # [K, M] -> [128 p, K//128 kt, nmt mt, 128 m]
        k = w.shape[0] // 128
        return np.ascontiguousarray(
            w.reshape(k, 128, nmt, 128).transpose(1, 0, 2, 3))

    def rep_slab(bh):  # [U] -> [16, U] replicated over batch rows
        return np.ascontiguousarray(np.repeat(bh[None, :], 16, axis=0))

    b2c = b2[0].copy()
    b2c[:2 * U] += b2[1][:2 * U]

    eb = np.where(enc_mask, 0.0, np.float32(NEG)).astype(np.float32)
    ebias = np.ascontiguousarray(np.repeat(eb[:, None, :], 128, axis=1))
    df = dec_mask.astype(np.float32)
    dfac = np.ascontiguousarray(np.repeat(df[:, None, :], 128, axis=1))

    common = {
        "x1t": x1t,
        "w1h": ktile(W1h),
        "w2x": lhstile(W2x, 24).astype(bf),
        "w2h": ktile(W2h),
        "b1h": rep_slab(b1[1][2 * U:]),
        "b2h": rep_slab(b2[1][2 * U:]),
        "b2ev": np.ascontiguousarray(b2c.reshape(24, 128).T),
        "h0": np.zeros((128, 128), np.float32),
        "encn": enc,
        "enct": np.ascontiguousarray(
            enc.transpose(0, 2, 1).reshape(B, 8, 128, TE)),
        "wa": lhstile(Wa, 8),
        "wc": lhstile(Wc, 8).astype(bf),
        "ebias": ebias,
        "dfac": dfac,
    }
    in_maps = []
    for i in range(NCORES):
        m = dict(common)
        m["fcw"] = ktile(fcW[:, i * VS:(i + 1) * VS]).astype(bf)
        m["fcb"] = np.ascontiguousarray(
            np.repeat(fcb[None, i * VS:(i + 1) * VS], 128, axis=0))
        in_maps.append(m)
    return in_maps


def _run(inputs, t_steps=TD, trace=False, debug=False):
    from concourse.bass_utils import run_bass_kernel_spmd

    key = ("nc", t_steps, debug)
    if key not in _CACHE:
        _CACHE[key] = _build(t_steps, debug=debug)
    nc = _CACHE[key]
    in_maps = _prep_host(inputs, t_steps)
    res = run_bass_kernel_spmd(nc, in_maps, core_ids=list(range(NCORES)),
                               trace=trace)

    def unperm(lg):  # [m, (t' b), v] -> [B, t, v]
        nm = lg.shape[0]
        return np.ascontiguousarray(
            lg.reshape(nm, 8, 16, VS).transpose(2, 0, 1, 3)
            .reshape(16, nm * 8, VS))

    logits = np.concatenate(
        [unperm(res.results[i]["logits"]) for i in range(NCORES)], axis=-1)
    attn = res.results[0]["attn"]
    h1 = res.results[0]["h1o"]
    h2 = res.results[0]["h2o"]
    return (logits, attn, h1, h2), res


def kernel(**inputs):
    out, _ = _run(inputs)
    return out
